# revision 1
# baseline (speedup 1.0000x reference)
"""Trainium2 Bass kernel for nn_Experts (grouped MoE expert MLP).

Computes, for each of 8 experts e:
    h   = x_e @ w0_e.T          # [2048,1024] @ [1024,4096] -> [2048,4096]
    g   = gelu_exact(h)
    out = g @ w3_e.T            # [2048,4096] @ [4096,1024] -> [2048,1024]
then masks unpopular experts with zero gating activity (output_tensor).

Sharding: expert-parallel, 1 expert per NeuronCore across 8 cores (SPMD —
one compiled NEFF, per-core input data).

Layout strategy: all operands are pre-transposed on the host into
contraction-major ("K-major") layouts so the device kernel needs no
transposes at all:
    xT  [128, 8, 2048]  (d%128, d//128, t)   bf16
    w0T [128, 8, 4096]  (d%128, d//128, f)   bf16
    w3T [128, 32, 1024] (f%128, f//128, d)   bf16
GEMM1 produces hT tiles [f=128, t] in PSUM, GELU moves them to SBUF as bf16,
and those tiles are directly the lhsT operand of GEMM2 (contraction over f),
whose PSUM output [t=128, d] accumulates over all 32 f-chunks and lands in
the natural [t, d] layout of the output.
"""

import numpy as np
import ml_dtypes

T = 2048      # tokens (capacity) per expert
D = 1024      # hidden
F = 4096      # ffn
P = 128       # partitions
TB = 256      # token block (GEMM1 moving free dim)
NTB = T // TB
DC = D // P   # 8 d-chunks (GEMM1 contraction)
FC = F // P   # 32 f-chunks (GEMM2 contraction)
DW = 512      # GEMM2 output free-dim chunk
NUM_LOCAL = 4
N_CORES = 8

_cache = {}


def _build_nc():
    import sys
    if "/opt/trn_rl_repo" not in sys.path:
        sys.path.insert(0, "/opt/trn_rl_repo")
    import concourse.bass as bass
    import concourse.tile as tile
    import concourse.mybir as mybir
    from concourse import bacc

    bf16 = mybir.dt.bfloat16
    f32 = mybir.dt.float32
    AFT = mybir.ActivationFunctionType

    nc = bacc.Bacc(
        "TRN2",
        target_bir_lowering=False,
        debug=False,
        enable_asserts=True,
        num_devices=N_CORES,
    )

    xT = nc.dram_tensor("xT", [P, DC, T], bf16, kind="ExternalInput").ap()
    w0T = nc.dram_tensor("w0T", [P, DC, F], bf16, kind="ExternalInput").ap()
    w3T = nc.dram_tensor("w3T", [P, FC, D], bf16, kind="ExternalInput").ap()
    out = nc.dram_tensor("out", [T, D], f32, kind="ExternalOutput").ap()

    with tile.TileContext(nc) as tc:
        with (
            tc.tile_pool(name="weights", bufs=1) as wpool,
            tc.tile_pool(name="gelu", bufs=4) as gpool,
            tc.tile_pool(name="ostage", bufs=4) as opool,
            tc.tile_pool(name="hps", bufs=2, space="PSUM") as hpsum,
            tc.tile_pool(name="ops", bufs=4, space="PSUM") as opsum,
        ):
            x_sb = wpool.tile([P, DC, T], bf16, name="x_sb", tag="x_sb")
            w0_sb = wpool.tile([P, DC, F], bf16, name="w0_sb", tag="w0_sb")
            w3_sb = wpool.tile([P, FC, D], bf16, name="w3_sb", tag="w3_sb")

            # Load x and w0 first (first h-tile needs ALL d-chunks of both);
            # w3 f-chunks stream in behind.
            for dc in range(DC):
                nc.sync.dma_start(x_sb[:, dc], xT[:, dc])
            for dc in range(DC):
                nc.sync.dma_start(w0_sb[:, dc], w0T[:, dc])
            for fc in range(FC):
                nc.sync.dma_start(w3_sb[:, fc], w3T[:, fc])

            for tb in range(NTB):
                o_ps = [
                    opsum.tile([P, DW], f32, name=f"o_ps_{tb}_{i}", tag="o_ps")
                    for i in range(4)
                ]
                for fc in range(FC):
                    h_ps = hpsum.tile([P, TB], f32, name=f"h_ps_{tb}_{fc}", tag="h_ps")
                    for dc in range(DC):
                        nc.tensor.matmul(
                            h_ps[:],
                            w0_sb[:, dc, fc * P:(fc + 1) * P],
                            x_sb[:, dc, tb * TB:(tb + 1) * TB],
                            start=(dc == 0),
                            stop=(dc == DC - 1),
                        )
                    g_sb = gpool.tile([P, TB], bf16, name=f"g_{tb}_{fc}", tag="g")
                    nc.scalar.activation(g_sb[:], h_ps[:], AFT.Gelu)
                    for ts in range(2):
                        for dc2 in range(2):
                            nc.tensor.matmul(
                                o_ps[ts * 2 + dc2][:],
                                g_sb[:, ts * P:(ts + 1) * P],
                                w3_sb[:, fc, dc2 * DW:(dc2 + 1) * DW],
                                start=(fc == 0),
                                stop=(fc == FC - 1),
                            )
                for ts in range(2):
                    for dc2 in range(2):
                        o_sb = opool.tile([P, DW], f32, name=f"o_sb_{tb}_{ts}_{dc2}",
                                          tag="o_sb")
                        nc.vector.tensor_copy(o_sb[:], o_ps[ts * 2 + dc2][:])
                        nc.sync.dma_start(
                            out[tb * TB + ts * P: tb * TB + (ts + 1) * P,
                                dc2 * DW:(dc2 + 1) * DW],
                            o_sb[:],
                        )

    nc.compile()
    return nc


def _get_nc():
    if "nc" not in _cache:
        _cache["nc"] = _build_nc()
    return _cache["nc"]


def kernel(**inputs):
    import sys
    if "/opt/trn_rl_repo" not in sys.path:
        sys.path.insert(0, "/opt/trn_rl_repo")
    from concourse import bass_utils

    output_tensor = np.asarray(inputs["output_tensor"], dtype=np.float32)  # [1, 8]
    x = np.asarray(inputs["inputs"], dtype=np.float32)   # [1, 8, 2048, 1024]
    w0 = np.asarray(inputs["w0"], dtype=np.float32)      # [8, 4096, 1024]
    w3 = np.asarray(inputs["w3"], dtype=np.float32)      # [8, 1024, 4096]

    bf = ml_dtypes.bfloat16
    in_maps = []
    for e in range(N_CORES):
        # xT: [d, t] -> [128, 8, 2048]
        xTe = x[0, e].T.reshape(DC, P, T).transpose(1, 0, 2)
        # w0T: [d, f] -> [128, 8, 4096]
        w0Te = w0[e].T.reshape(DC, P, F).transpose(1, 0, 2)
        # w3T: [f, d] -> [128, 32, 1024]
        w3Te = w3[e].T.reshape(FC, P, D).transpose(1, 0, 2)
        in_maps.append({
            "xT": np.ascontiguousarray(xTe).astype(bf),
            "w0T": np.ascontiguousarray(w0Te).astype(bf),
            "w3T": np.ascontiguousarray(w3Te).astype(bf),
        })

    nc = _get_nc()
    res = bass_utils.run_bass_kernel_spmd(nc, in_maps, core_ids=list(range(N_CORES)))
    out_full = np.stack([res.results[e]["out"] for e in range(N_CORES)])[None]

    # unpopular experts with zero gating activity produce zeros
    unpop = output_tensor[:, NUM_LOCAL:].sum(axis=0) != 0
    mask = np.concatenate([np.ones(NUM_LOCAL, dtype=bool), unpop])
    out_full = out_full * mask[None, :, None, None].astype(np.float32)
    return out_full.astype(np.float32)


# revision 12
# speedup vs baseline: 36.2012x; 36.2012x over previous
"""Trainium2 Bass kernel for nn_Experts (grouped MoE expert MLP).

Computes, for each of 8 experts e:
    h   = x_e @ w0_e.T          # [2048,1024] @ [1024,4096] -> [2048,4096]
    g   = gelu_exact(h)
    out = g @ w3_e.T            # [2048,4096] @ [4096,1024] -> [2048,1024]
then masks unpopular experts with zero gating activity (output_tensor).

Sharding: expert-parallel, 1 expert per NeuronCore across 8 cores (SPMD —
one compiled NEFF, per-core input data).

Layout strategy: all operands are pre-transposed on the host into
contraction-major ("K-major") layouts so the device kernel needs no
transposes at all:
    xT  [128, 8, 2048]  (d%128, d//128, t)   bf16
    w0T [128, 8, 4096]  (d%128, d//128, f)   bf16
    w3T [128, 32, 1024] (f%128, f//128, d)   bf16
GEMM1 produces hT tiles [f=128, t] in PSUM, GELU moves them to SBUF as bf16,
and those tiles are directly the lhsT operand of GEMM2 (contraction over f),
whose PSUM output [t=128, d] accumulates over all 32 f-chunks and lands in
the natural [t, d] layout of the output.
"""

import numpy as np
import ml_dtypes

T = 2048      # tokens (capacity) per expert
D = 1024      # hidden
F = 4096      # ffn
P = 128       # partitions
TB = 256      # token block (GEMM1 moving free dim)
NTB = T // TB
DC = D // P   # 8 d-chunks (GEMM1 contraction)
FC = F // P   # 32 f-chunks (GEMM2 contraction)
DW = 512      # GEMM2 output free-dim chunk
NUM_LOCAL = 4
N_CORES = 8

_cache = {}


def _build_nc(
    tb_size=TB,          # token block
    x_split=1,           # extra splits of each x d-chunk DMA (along t)
    w0_split=1,          # extra splits of each w0 d-chunk DMA (along f)
    w3_group=1,          # f-chunks per w3 DMA
    g_bufs=4,
    h_bufs=2,
    o_sb_bufs=4,
    dma_scheme="tuned",  # "simple" | "tuned" (critical-prefix-first ordering)
    fcg=4,               # fc per w0/w3 DMA group in tuned scheme
    pipeline_o=True,     # issue GEMM2(fc) after GEMM1(fc+1) to hide gelu latency
):
    import sys
    if "/opt/trn_rl_repo" not in sys.path:
        sys.path.insert(0, "/opt/trn_rl_repo")
    import concourse.bass as bass
    import concourse.tile as tile
    import concourse.mybir as mybir
    from concourse import bacc

    bf16 = mybir.dt.bfloat16
    f32 = mybir.dt.float32
    AFT = mybir.ActivationFunctionType

    TBS = tb_size
    NTBS = T // TBS
    NTS = TBS // P       # t-subchunks per block (GEMM2 lhsT count)
    n_ops = NTS * 2      # out psum tiles per block ([t 128] x [d 512])

    nc = bacc.Bacc(
        "TRN2",
        target_bir_lowering=False,
        debug=False,
        enable_asserts=True,
        num_devices=N_CORES,
    )

    xT = nc.dram_tensor("xT", [P, DC, T], bf16, kind="ExternalInput").ap()
    w0T = nc.dram_tensor("w0T", [P, DC, F], bf16, kind="ExternalInput").ap()
    w3T = nc.dram_tensor("w3T", [P, FC, D], bf16, kind="ExternalInput").ap()
    out = nc.dram_tensor("out", [T, D], f32, kind="ExternalOutput").ap()

    with tile.TileContext(nc) as tc:
        with (
            tc.tile_pool(name="weights", bufs=1) as wpool,
            tc.tile_pool(name="gelu", bufs=g_bufs) as gpool,
            tc.tile_pool(name="ostage", bufs=o_sb_bufs) as opool,
            tc.tile_pool(name="hps", bufs=h_bufs, space="PSUM") as hpsum,
            tc.tile_pool(name="ops", bufs=n_ops, space="PSUM") as opsum,
        ):
            x_sb = wpool.tile([P, DC, T], bf16, name="x_sb", tag="x_sb")
            w0_sb = wpool.tile([P, DC, F], bf16, name="w0_sb", tag="w0_sb")
            w3_sb = wpool.tile([P, FC, D], bf16, name="w3_sb", tag="w3_sb")

            if dma_scheme == "simple":
                # Load x and w0 first (first h-tile needs ALL d-chunks of
                # both); w3 f-chunks stream in behind.
                for dc in range(DC):
                    for s in range(x_split):
                        w = T // x_split
                        nc.sync.dma_start(x_sb[:, dc, s * w:(s + 1) * w],
                                          xT[:, dc, s * w:(s + 1) * w])
                    for s in range(w0_split):
                        w = F // w0_split
                        nc.sync.dma_start(w0_sb[:, dc, s * w:(s + 1) * w],
                                          w0T[:, dc, s * w:(s + 1) * w])
                for g in range(FC // w3_group):
                    lo, hi = g * w3_group, (g + 1) * w3_group
                    nc.sync.dma_start(w3_sb[:, lo:hi], w3T[:, lo:hi])
            else:
                # Critical-prefix-first: x for tb0, then per-f-group w0 (all
                # d-chunks) and w3 interleaved in the order GEMM1/GEMM2
                # consume them, then the rest of x.
                for dc in range(DC):
                    nc.sync.dma_start(x_sb[:, dc, 0:TBS], xT[:, dc, 0:TBS])
                for g in range(FC // fcg):
                    flo, fhi = g * fcg * P, (g + 1) * fcg * P
                    for dc in range(DC):
                        nc.sync.dma_start(w0_sb[:, dc, flo:fhi],
                                          w0T[:, dc, flo:fhi])
                    nc.sync.dma_start(w3_sb[:, g * fcg:(g + 1) * fcg],
                                      w3T[:, g * fcg:(g + 1) * fcg])
                for tb in range(1, NTBS):
                    for dc in range(DC):
                        nc.sync.dma_start(
                            x_sb[:, dc, tb * TBS:(tb + 1) * TBS],
                            xT[:, dc, tb * TBS:(tb + 1) * TBS])

            for tb in range(NTBS):
                o_ps = [
                    opsum.tile([P, DW], f32, name=f"o_ps_{tb}_{i}", tag="o_ps")
                    for i in range(n_ops)
                ]

                def emit_o(fc, g_sb):
                    for ts in range(NTS):
                        for dc2 in range(2):
                            nc.tensor.matmul(
                                o_ps[ts * 2 + dc2][:],
                                g_sb[:, ts * P:(ts + 1) * P],
                                w3_sb[:, fc, dc2 * DW:(dc2 + 1) * DW],
                                start=(fc == 0),
                                stop=(fc == FC - 1),
                            )

                pending = None
                for fc in range(FC):
                    h_ps = hpsum.tile([P, TBS], f32, name=f"h_ps_{tb}_{fc}", tag="h_ps")
                    for dc in range(DC):
                        nc.tensor.matmul(
                            h_ps[:],
                            w0_sb[:, dc, fc * P:(fc + 1) * P],
                            x_sb[:, dc, tb * TBS:(tb + 1) * TBS],
                            start=(dc == 0),
                            stop=(dc == DC - 1),
                        )
                    g_sb = gpool.tile([P, TBS], bf16, name=f"g_{tb}_{fc}", tag="g")
                    nc.scalar.activation(g_sb[:], h_ps[:], AFT.Gelu)
                    if not pipeline_o:
                        emit_o(fc, g_sb)
                    else:
                        if pending is not None:
                            emit_o(*pending)
                        pending = (fc, g_sb)
                if pending is not None:
                    emit_o(*pending)

                for ts in range(NTS):
                    for dc2 in range(2):
                        o_sb = opool.tile([P, DW], f32, name=f"o_sb_{tb}_{ts}_{dc2}",
                                          tag="o_sb")
                        nc.vector.tensor_copy(o_sb[:], o_ps[ts * 2 + dc2][:])
                        nc.sync.dma_start(
                            out[tb * TBS + ts * P: tb * TBS + (ts + 1) * P,
                                dc2 * DW:(dc2 + 1) * DW],
                            o_sb[:],
                        )

    nc.compile()
    return nc


def _build_nc_v2(
    g_extra=0,           # extra gelu-tile slots beyond FC (lookahead into next block)
    h_bufs=2,
    o_ps_bufs=2,
    o_sb_bufs=3,
    x_bufs=2,
    fcg=4,               # fc per w0/w3 DMA group
    x_coarse=True,       # one DMA per x block vs per-dc
    w0_coarse=False,     # one DMA per w0 f-group vs per-dc
):
    """TB=512 two-phase variant: per 512-token block, phase A runs GEMM1+GELU
    for all 32 f-chunks (g tiles [128,512] bf16 stay in SBUF), phase B runs
    GEMM2 as 8 sequential PSUM accumulation groups (one [t=128, d=512] output
    tile each, contraction over all 32 f-chunks). x is streamed per-block
    instead of fully resident to stay under the SBUF cap."""
    import sys
    if "/opt/trn_rl_repo" not in sys.path:
        sys.path.insert(0, "/opt/trn_rl_repo")
    import concourse.tile as tile
    import concourse.mybir as mybir
    from concourse import bacc

    bf16 = mybir.dt.bfloat16
    f32 = mybir.dt.float32
    AFT = mybir.ActivationFunctionType

    TBS = 512
    NTBS = T // TBS      # 4
    NTS = TBS // P       # 4

    nc = bacc.Bacc(
        "TRN2",
        target_bir_lowering=False,
        debug=False,
        enable_asserts=True,
        num_devices=N_CORES,
    )

    xT = nc.dram_tensor("xT", [P, DC, T], bf16, kind="ExternalInput").ap()
    w0T = nc.dram_tensor("w0T", [P, DC, F], bf16, kind="ExternalInput").ap()
    w3T = nc.dram_tensor("w3T", [P, FC, D], bf16, kind="ExternalInput").ap()
    out = nc.dram_tensor("out", [T, D], f32, kind="ExternalOutput").ap()

    with tile.TileContext(nc) as tc:
        with (
            tc.tile_pool(name="weights", bufs=1) as wpool,
            tc.tile_pool(name="xin", bufs=x_bufs) as xpool,
            tc.tile_pool(name="gelu", bufs=FC + g_extra) as gpool,
            tc.tile_pool(name="ostage", bufs=o_sb_bufs) as opool,
            tc.tile_pool(name="hps", bufs=h_bufs, space="PSUM") as hpsum,
            tc.tile_pool(name="ops", bufs=o_ps_bufs, space="PSUM") as opsum,
        ):
            w0_sb = wpool.tile([P, DC, F], bf16, name="w0_sb", tag="w0_sb")
            w3_sb = wpool.tile([P, FC, D], bf16, name="w3_sb", tag="w3_sb")

            x_tiles = {}
            def load_x(tb):
                xt = xpool.tile([P, DC, TBS], bf16, name=f"x_{tb}", tag="x")
                if x_coarse:
                    nc.sync.dma_start(xt[:], xT[:, :, tb * TBS:(tb + 1) * TBS])
                else:
                    for dc in range(DC):
                        nc.sync.dma_start(xt[:, dc],
                                          xT[:, dc, tb * TBS:(tb + 1) * TBS])
                x_tiles[tb] = xt

            # critical prefix: x[tb0], then w0/w3 by f-group in consumption order
            load_x(0)
            for g in range(FC // fcg):
                flo, fhi = g * fcg * P, (g + 1) * fcg * P
                if w0_coarse:
                    nc.sync.dma_start(w0_sb[:, :, flo:fhi], w0T[:, :, flo:fhi])
                else:
                    for dc in range(DC):
                        nc.sync.dma_start(w0_sb[:, dc, flo:fhi],
                                          w0T[:, dc, flo:fhi])
                nc.sync.dma_start(w3_sb[:, g * fcg:(g + 1) * fcg],
                                  w3T[:, g * fcg:(g + 1) * fcg])

            for tb in range(NTBS):
                if tb + 1 < NTBS:
                    load_x(tb + 1)
                xt = x_tiles.pop(tb)
                # phase A: GEMM1 + GELU for all fc
                g_tiles = []
                for fc in range(FC):
                    h_ps = hpsum.tile([P, TBS], f32, name=f"h_{tb}_{fc}", tag="h_ps")
                    for dc in range(DC):
                        nc.tensor.matmul(
                            h_ps[:],
                            w0_sb[:, dc, fc * P:(fc + 1) * P],
                            xt[:, dc],
                            start=(dc == 0),
                            stop=(dc == DC - 1),
                        )
                    g_sb = gpool.tile([P, TBS], bf16, name=f"g_{tb}_{fc}", tag="g")
                    nc.scalar.activation(g_sb[:], h_ps[:], AFT.Gelu)
                    g_tiles.append(g_sb)
                # phase B: GEMM2, one [t=128, d=512] accumulation group at a time
                for ts in range(NTS):
                    for dc2 in range(2):
                        o_ps = opsum.tile([P, DW], f32, name=f"o_{tb}_{ts}_{dc2}",
                                          tag="o_ps")
                        for fc in range(FC):
                            nc.tensor.matmul(
                                o_ps[:],
                                g_tiles[fc][:, ts * P:(ts + 1) * P],
                                w3_sb[:, fc, dc2 * DW:(dc2 + 1) * DW],
                                start=(fc == 0),
                                stop=(fc == FC - 1),
                            )
                        o_sb = opool.tile([P, DW], f32, name=f"os_{tb}_{ts}_{dc2}",
                                          tag="o_sb")
                        nc.vector.tensor_copy(o_sb[:], o_ps[:])
                        nc.sync.dma_start(
                            out[tb * TBS + ts * P: tb * TBS + (ts + 1) * P,
                                dc2 * DW:(dc2 + 1) * DW],
                            o_sb[:],
                        )

    nc.compile()
    return nc


def _get_nc():
    if "nc" not in _cache:
        import os
        variant = os.environ.get("KERNEL_VARIANT", "v2")
        _cache["nc"] = _build_nc_v2() if variant == "v2" else _build_nc()
    return _cache["nc"]


def kernel(**inputs):
    import sys
    if "/opt/trn_rl_repo" not in sys.path:
        sys.path.insert(0, "/opt/trn_rl_repo")
    from concourse import bass_utils

    output_tensor = np.asarray(inputs["output_tensor"], dtype=np.float32)  # [1, 8]
    x = np.asarray(inputs["inputs"], dtype=np.float32)   # [1, 8, 2048, 1024]
    w0 = np.asarray(inputs["w0"], dtype=np.float32)      # [8, 4096, 1024]
    w3 = np.asarray(inputs["w3"], dtype=np.float32)      # [8, 1024, 4096]

    bf = ml_dtypes.bfloat16
    in_maps = []
    for e in range(N_CORES):
        # cast to bf16 first (halves bytes moved by the transposes)
        xe = x[0, e].astype(bf)     # [t, d]
        w0e = w0[e].astype(bf)      # [f, d]
        w3e = w3[e].astype(bf)      # [d, f]
        # xT: [d, t] -> [128, 8, 2048]; w0T: [d, f] -> [128, 8, 4096];
        # w3T: [f, d] -> [128, 32, 1024]  (partition = contraction dim % 128)
        in_maps.append({
            "xT": np.ascontiguousarray(xe.T.reshape(DC, P, T).transpose(1, 0, 2)),
            "w0T": np.ascontiguousarray(w0e.T.reshape(DC, P, F).transpose(1, 0, 2)),
            "w3T": np.ascontiguousarray(w3e.T.reshape(FC, P, D).transpose(1, 0, 2)),
        })

    nc = _get_nc()
    res = bass_utils.run_bass_kernel_spmd(nc, in_maps, core_ids=list(range(N_CORES)))
    out_full = np.stack([res.results[e]["out"] for e in range(N_CORES)])[None]

    # unpopular experts with zero gating activity produce zeros
    unpop = output_tensor[:, NUM_LOCAL:].sum(axis=0) != 0
    mask = np.concatenate([np.ones(NUM_LOCAL, dtype=bool), unpop])
    out_full = out_full * mask[None, :, None, None].astype(np.float32)
    return out_full.astype(np.float32)


# revision 15
# speedup vs baseline: 36.3073x; 1.0029x over previous
"""Trainium2 Bass kernel for nn_Experts (grouped MoE expert MLP).

Computes, for each of 8 experts e:
    h   = x_e @ w0_e.T          # [2048,1024] @ [1024,4096] -> [2048,4096]
    g   = gelu_exact(h)
    out = g @ w3_e.T            # [2048,4096] @ [4096,1024] -> [2048,1024]
then masks unpopular experts with zero gating activity (output_tensor).

Sharding: expert-parallel, 1 expert per NeuronCore across 8 cores (SPMD —
one compiled NEFF, per-core input data).

Layout strategy: all operands are pre-transposed on the host into
contraction-major ("K-major") layouts so the device kernel needs no
transposes at all:
    xT  [128, 8, 2048]  (d%128, d//128, t)   bf16
    w0T [128, 8, 4096]  (d%128, d//128, f)   bf16
    w3T [128, 32, 1024] (f%128, f//128, d)   bf16
GEMM1 produces hT tiles [f=128, t] in PSUM, GELU moves them to SBUF as bf16,
and those tiles are directly the lhsT operand of GEMM2 (contraction over f),
whose PSUM output [t=128, d] accumulates over all 32 f-chunks and lands in
the natural [t, d] layout of the output.
"""

import numpy as np
import ml_dtypes

T = 2048      # tokens (capacity) per expert
D = 1024      # hidden
F = 4096      # ffn
P = 128       # partitions
TB = 256      # token block (GEMM1 moving free dim)
NTB = T // TB
DC = D // P   # 8 d-chunks (GEMM1 contraction)
FC = F // P   # 32 f-chunks (GEMM2 contraction)
DW = 512      # GEMM2 output free-dim chunk
NUM_LOCAL = 4
N_CORES = 8

_cache = {}


def _build_nc(
    tb_size=TB,          # token block
    x_split=1,           # extra splits of each x d-chunk DMA (along t)
    w0_split=1,          # extra splits of each w0 d-chunk DMA (along f)
    w3_group=1,          # f-chunks per w3 DMA
    g_bufs=4,
    h_bufs=2,
    o_sb_bufs=4,
    dma_scheme="tuned",  # "simple" | "tuned" (critical-prefix-first ordering)
    fcg=4,               # fc per w0/w3 DMA group in tuned scheme
    pipeline_o=True,     # issue GEMM2(fc) after GEMM1(fc+1) to hide gelu latency
):
    import sys
    if "/opt/trn_rl_repo" not in sys.path:
        sys.path.insert(0, "/opt/trn_rl_repo")
    import concourse.bass as bass
    import concourse.tile as tile
    import concourse.mybir as mybir
    from concourse import bacc

    bf16 = mybir.dt.bfloat16
    f32 = mybir.dt.float32
    AFT = mybir.ActivationFunctionType

    TBS = tb_size
    NTBS = T // TBS
    NTS = TBS // P       # t-subchunks per block (GEMM2 lhsT count)
    n_ops = NTS * 2      # out psum tiles per block ([t 128] x [d 512])

    nc = bacc.Bacc(
        "TRN2",
        target_bir_lowering=False,
        debug=False,
        enable_asserts=True,
        num_devices=N_CORES,
    )

    xT = nc.dram_tensor("xT", [P, DC, T], bf16, kind="ExternalInput").ap()
    w0T = nc.dram_tensor("w0T", [P, DC, F], bf16, kind="ExternalInput").ap()
    w3T = nc.dram_tensor("w3T", [P, FC, D], bf16, kind="ExternalInput").ap()
    out = nc.dram_tensor("out", [T, D], f32, kind="ExternalOutput").ap()

    with tile.TileContext(nc) as tc:
        with (
            tc.tile_pool(name="weights", bufs=1) as wpool,
            tc.tile_pool(name="gelu", bufs=g_bufs) as gpool,
            tc.tile_pool(name="ostage", bufs=o_sb_bufs) as opool,
            tc.tile_pool(name="hps", bufs=h_bufs, space="PSUM") as hpsum,
            tc.tile_pool(name="ops", bufs=n_ops, space="PSUM") as opsum,
        ):
            x_sb = wpool.tile([P, DC, T], bf16, name="x_sb", tag="x_sb")
            w0_sb = wpool.tile([P, DC, F], bf16, name="w0_sb", tag="w0_sb")
            w3_sb = wpool.tile([P, FC, D], bf16, name="w3_sb", tag="w3_sb")

            if dma_scheme == "simple":
                # Load x and w0 first (first h-tile needs ALL d-chunks of
                # both); w3 f-chunks stream in behind.
                for dc in range(DC):
                    for s in range(x_split):
                        w = T // x_split
                        nc.sync.dma_start(x_sb[:, dc, s * w:(s + 1) * w],
                                          xT[:, dc, s * w:(s + 1) * w])
                    for s in range(w0_split):
                        w = F // w0_split
                        nc.sync.dma_start(w0_sb[:, dc, s * w:(s + 1) * w],
                                          w0T[:, dc, s * w:(s + 1) * w])
                for g in range(FC // w3_group):
                    lo, hi = g * w3_group, (g + 1) * w3_group
                    nc.sync.dma_start(w3_sb[:, lo:hi], w3T[:, lo:hi])
            else:
                # Critical-prefix-first: x for tb0, then per-f-group w0 (all
                # d-chunks) and w3 interleaved in the order GEMM1/GEMM2
                # consume them, then the rest of x.
                for dc in range(DC):
                    nc.sync.dma_start(x_sb[:, dc, 0:TBS], xT[:, dc, 0:TBS])
                for g in range(FC // fcg):
                    flo, fhi = g * fcg * P, (g + 1) * fcg * P
                    for dc in range(DC):
                        nc.sync.dma_start(w0_sb[:, dc, flo:fhi],
                                          w0T[:, dc, flo:fhi])
                    nc.sync.dma_start(w3_sb[:, g * fcg:(g + 1) * fcg],
                                      w3T[:, g * fcg:(g + 1) * fcg])
                for tb in range(1, NTBS):
                    for dc in range(DC):
                        nc.sync.dma_start(
                            x_sb[:, dc, tb * TBS:(tb + 1) * TBS],
                            xT[:, dc, tb * TBS:(tb + 1) * TBS])

            for tb in range(NTBS):
                o_ps = [
                    opsum.tile([P, DW], f32, name=f"o_ps_{tb}_{i}", tag="o_ps")
                    for i in range(n_ops)
                ]

                def emit_o(fc, g_sb):
                    for ts in range(NTS):
                        for dc2 in range(2):
                            nc.tensor.matmul(
                                o_ps[ts * 2 + dc2][:],
                                g_sb[:, ts * P:(ts + 1) * P],
                                w3_sb[:, fc, dc2 * DW:(dc2 + 1) * DW],
                                start=(fc == 0),
                                stop=(fc == FC - 1),
                            )

                pending = None
                for fc in range(FC):
                    h_ps = hpsum.tile([P, TBS], f32, name=f"h_ps_{tb}_{fc}", tag="h_ps")
                    for dc in range(DC):
                        nc.tensor.matmul(
                            h_ps[:],
                            w0_sb[:, dc, fc * P:(fc + 1) * P],
                            x_sb[:, dc, tb * TBS:(tb + 1) * TBS],
                            start=(dc == 0),
                            stop=(dc == DC - 1),
                        )
                    g_sb = gpool.tile([P, TBS], bf16, name=f"g_{tb}_{fc}", tag="g")
                    nc.scalar.activation(g_sb[:], h_ps[:], AFT.Gelu)
                    if not pipeline_o:
                        emit_o(fc, g_sb)
                    else:
                        if pending is not None:
                            emit_o(*pending)
                        pending = (fc, g_sb)
                if pending is not None:
                    emit_o(*pending)

                for ts in range(NTS):
                    for dc2 in range(2):
                        o_sb = opool.tile([P, DW], f32, name=f"o_sb_{tb}_{ts}_{dc2}",
                                          tag="o_sb")
                        nc.vector.tensor_copy(o_sb[:], o_ps[ts * 2 + dc2][:])
                        nc.sync.dma_start(
                            out[tb * TBS + ts * P: tb * TBS + (ts + 1) * P,
                                dc2 * DW:(dc2 + 1) * DW],
                            o_sb[:],
                        )

    nc.compile()
    return nc


def _build_nc_v2(
    g_extra=0,           # extra gelu-tile slots beyond FC (lookahead into next block)
    h_bufs=2,
    o_ps_bufs=2,
    o_sb_bufs=3,
    x_bufs=2,
    fcg=4,               # fc per w0/w3 DMA group
    x_coarse=True,       # one DMA per x block vs per-dc
    w0_coarse=False,     # one DMA per w0 f-group vs per-dc
    warmup_mms=16,       # scratch matmuls issued before the real work so the
                         # PE rides out the HAM cold-clock window during the
                         # initial DMA wait instead of during real matmuls
):
    """TB=512 two-phase variant: per 512-token block, phase A runs GEMM1+GELU
    for all 32 f-chunks (g tiles [128,512] bf16 stay in SBUF), phase B runs
    GEMM2 as 8 sequential PSUM accumulation groups (one [t=128, d=512] output
    tile each, contraction over all 32 f-chunks). x is streamed per-block
    instead of fully resident to stay under the SBUF cap."""
    import sys
    if "/opt/trn_rl_repo" not in sys.path:
        sys.path.insert(0, "/opt/trn_rl_repo")
    import concourse.tile as tile
    import concourse.mybir as mybir
    from concourse import bacc

    bf16 = mybir.dt.bfloat16
    f32 = mybir.dt.float32
    AFT = mybir.ActivationFunctionType

    TBS = 512
    NTBS = T // TBS      # 4
    NTS = TBS // P       # 4

    nc = bacc.Bacc(
        "TRN2",
        target_bir_lowering=False,
        debug=False,
        enable_asserts=True,
        num_devices=N_CORES,
    )

    xT = nc.dram_tensor("xT", [P, DC, T], bf16, kind="ExternalInput").ap()
    w0T = nc.dram_tensor("w0T", [P, DC, F], bf16, kind="ExternalInput").ap()
    w3T = nc.dram_tensor("w3T", [P, FC, D], bf16, kind="ExternalInput").ap()
    out = nc.dram_tensor("out", [T, D], f32, kind="ExternalOutput").ap()

    with tile.TileContext(nc) as tc:
        with (
            tc.tile_pool(name="weights", bufs=1) as wpool,
            tc.tile_pool(name="xin", bufs=x_bufs) as xpool,
            tc.tile_pool(name="gelu", bufs=FC + g_extra) as gpool,
            tc.tile_pool(name="ostage", bufs=o_sb_bufs) as opool,
            tc.tile_pool(name="hps", bufs=h_bufs, space="PSUM") as hpsum,
            tc.tile_pool(name="ops", bufs=o_ps_bufs, space="PSUM") as opsum,
        ):
            w0_sb = wpool.tile([P, DC, F], bf16, name="w0_sb", tag="w0_sb")
            w3_sb = wpool.tile([P, FC, D], bf16, name="w3_sb", tag="w3_sb")

            x_tiles = {}
            def load_x(tb):
                xt = xpool.tile([P, DC, TBS], bf16, name=f"x_{tb}", tag="x")
                if x_coarse:
                    nc.sync.dma_start(xt[:], xT[:, :, tb * TBS:(tb + 1) * TBS])
                else:
                    for dc in range(DC):
                        nc.sync.dma_start(xt[:, dc],
                                          xT[:, dc, tb * TBS:(tb + 1) * TBS])
                x_tiles[tb] = xt

            if warmup_mms:
                with (
                    tc.tile_pool(name="warm", bufs=1) as warmpool,
                    tc.tile_pool(name="warmps", bufs=1, space="PSUM") as warmpsum,
                ):
                    wsrc = warmpool.tile([P, DW], bf16, name="wsrc", tag="wsrc")
                    wps = warmpsum.tile([P, DW], f32, name="wps", tag="wps")
                    nc.gpsimd.memset(wsrc[:], 0.0)
                    for i in range(warmup_mms):
                        nc.tensor.matmul(wps[:], wsrc[:, :P], wsrc[:],
                                         start=(i == 0), stop=(i == warmup_mms - 1))

            # critical prefix: x[tb0], then w0/w3 by f-group in consumption order
            load_x(0)
            for g in range(FC // fcg):
                flo, fhi = g * fcg * P, (g + 1) * fcg * P
                if w0_coarse:
                    nc.sync.dma_start(w0_sb[:, :, flo:fhi], w0T[:, :, flo:fhi])
                else:
                    for dc in range(DC):
                        nc.sync.dma_start(w0_sb[:, dc, flo:fhi],
                                          w0T[:, dc, flo:fhi])
                nc.sync.dma_start(w3_sb[:, g * fcg:(g + 1) * fcg],
                                  w3T[:, g * fcg:(g + 1) * fcg])

            for tb in range(NTBS):
                if tb + 1 < NTBS:
                    load_x(tb + 1)
                xt = x_tiles.pop(tb)
                # phase A: GEMM1 + GELU for all fc
                g_tiles = []
                for fc in range(FC):
                    h_ps = hpsum.tile([P, TBS], f32, name=f"h_{tb}_{fc}", tag="h_ps")
                    for dc in range(DC):
                        nc.tensor.matmul(
                            h_ps[:],
                            w0_sb[:, dc, fc * P:(fc + 1) * P],
                            xt[:, dc],
                            start=(dc == 0),
                            stop=(dc == DC - 1),
                        )
                    g_sb = gpool.tile([P, TBS], bf16, name=f"g_{tb}_{fc}", tag="g")
                    nc.scalar.activation(g_sb[:], h_ps[:], AFT.Gelu)
                    g_tiles.append(g_sb)
                # phase B: GEMM2, one [t=128, d=512] accumulation group at a time
                for ts in range(NTS):
                    for dc2 in range(2):
                        o_ps = opsum.tile([P, DW], f32, name=f"o_{tb}_{ts}_{dc2}",
                                          tag="o_ps")
                        for fc in range(FC):
                            nc.tensor.matmul(
                                o_ps[:],
                                g_tiles[fc][:, ts * P:(ts + 1) * P],
                                w3_sb[:, fc, dc2 * DW:(dc2 + 1) * DW],
                                start=(fc == 0),
                                stop=(fc == FC - 1),
                            )
                        o_sb = opool.tile([P, DW], f32, name=f"os_{tb}_{ts}_{dc2}",
                                          tag="o_sb")
                        nc.vector.tensor_copy(o_sb[:], o_ps[:])
                        nc.sync.dma_start(
                            out[tb * TBS + ts * P: tb * TBS + (ts + 1) * P,
                                dc2 * DW:(dc2 + 1) * DW],
                            o_sb[:],
                        )

    nc.compile()
    return nc


def _get_nc():
    if "nc" not in _cache:
        import os
        variant = os.environ.get("KERNEL_VARIANT", "v2")
        _cache["nc"] = _build_nc_v2() if variant == "v2" else _build_nc()
    return _cache["nc"]


def kernel(**inputs):
    import sys
    if "/opt/trn_rl_repo" not in sys.path:
        sys.path.insert(0, "/opt/trn_rl_repo")
    from concourse import bass_utils

    output_tensor = np.asarray(inputs["output_tensor"], dtype=np.float32)  # [1, 8]
    x = np.asarray(inputs["inputs"], dtype=np.float32)   # [1, 8, 2048, 1024]
    w0 = np.asarray(inputs["w0"], dtype=np.float32)      # [8, 4096, 1024]
    w3 = np.asarray(inputs["w3"], dtype=np.float32)      # [8, 1024, 4096]

    bf = ml_dtypes.bfloat16
    in_maps = []
    for e in range(N_CORES):
        # cast to bf16 first (halves bytes moved by the transposes)
        xe = x[0, e].astype(bf)     # [t, d]
        w0e = w0[e].astype(bf)      # [f, d]
        w3e = w3[e].astype(bf)      # [d, f]
        # xT: [d, t] -> [128, 8, 2048]; w0T: [d, f] -> [128, 8, 4096];
        # w3T: [f, d] -> [128, 32, 1024]  (partition = contraction dim % 128)
        in_maps.append({
            "xT": np.ascontiguousarray(xe.T.reshape(DC, P, T).transpose(1, 0, 2)),
            "w0T": np.ascontiguousarray(w0e.T.reshape(DC, P, F).transpose(1, 0, 2)),
            "w3T": np.ascontiguousarray(w3e.T.reshape(FC, P, D).transpose(1, 0, 2)),
        })

    nc = _get_nc()
    res = bass_utils.run_bass_kernel_spmd(nc, in_maps, core_ids=list(range(N_CORES)))
    out_full = np.stack([res.results[e]["out"] for e in range(N_CORES)])[None]

    # unpopular experts with zero gating activity produce zeros
    unpop = output_tensor[:, NUM_LOCAL:].sum(axis=0) != 0
    mask = np.concatenate([np.ones(NUM_LOCAL, dtype=bool), unpop])
    out_full = out_full * mask[None, :, None, None].astype(np.float32)
    return out_full.astype(np.float32)


# revision 17
# speedup vs baseline: 36.5438x; 1.0065x over previous
"""Trainium2 Bass kernel for nn_Experts (grouped MoE expert MLP).

Computes, for each of 8 experts e:
    h   = x_e @ w0_e.T          # [2048,1024] @ [1024,4096] -> [2048,4096]
    g   = gelu_exact(h)
    out = g @ w3_e.T            # [2048,4096] @ [4096,1024] -> [2048,1024]
then masks unpopular experts with zero gating activity (output_tensor).

Sharding: expert-parallel, 1 expert per NeuronCore across 8 cores (SPMD —
one compiled NEFF, per-core input data).

Layout strategy: all operands are pre-transposed on the host into
contraction-major ("K-major") layouts so the device kernel needs no
transposes at all:
    xT  [128, 8, 2048]  (d%128, d//128, t)   bf16
    w0T [128, 8, 4096]  (d%128, d//128, f)   bf16
    w3T [128, 32, 1024] (f%128, f//128, d)   bf16
GEMM1 produces hT tiles [f=128, t] in PSUM, GELU moves them to SBUF as bf16,
and those tiles are directly the lhsT operand of GEMM2 (contraction over f),
whose PSUM output [t=128, d] accumulates over all 32 f-chunks and lands in
the natural [t, d] layout of the output.
"""

import numpy as np
import ml_dtypes

T = 2048      # tokens (capacity) per expert
D = 1024      # hidden
F = 4096      # ffn
P = 128       # partitions
TB = 256      # token block (GEMM1 moving free dim)
NTB = T // TB
DC = D // P   # 8 d-chunks (GEMM1 contraction)
FC = F // P   # 32 f-chunks (GEMM2 contraction)
DW = 512      # GEMM2 output free-dim chunk
NUM_LOCAL = 4
N_CORES = 8

_cache = {}


def _build_nc(
    tb_size=TB,          # token block
    x_split=1,           # extra splits of each x d-chunk DMA (along t)
    w0_split=1,          # extra splits of each w0 d-chunk DMA (along f)
    w3_group=1,          # f-chunks per w3 DMA
    g_bufs=4,
    h_bufs=2,
    o_sb_bufs=4,
    dma_scheme="tuned",  # "simple" | "tuned" (critical-prefix-first ordering)
    fcg=4,               # fc per w0/w3 DMA group in tuned scheme
    pipeline_o=True,     # issue GEMM2(fc) after GEMM1(fc+1) to hide gelu latency
):
    import sys
    if "/opt/trn_rl_repo" not in sys.path:
        sys.path.insert(0, "/opt/trn_rl_repo")
    import concourse.bass as bass
    import concourse.tile as tile
    import concourse.mybir as mybir
    from concourse import bacc

    bf16 = mybir.dt.bfloat16
    f32 = mybir.dt.float32
    AFT = mybir.ActivationFunctionType

    TBS = tb_size
    NTBS = T // TBS
    NTS = TBS // P       # t-subchunks per block (GEMM2 lhsT count)
    n_ops = NTS * 2      # out psum tiles per block ([t 128] x [d 512])

    nc = bacc.Bacc(
        "TRN2",
        target_bir_lowering=False,
        debug=False,
        enable_asserts=True,
        num_devices=N_CORES,
    )

    xT = nc.dram_tensor("xT", [P, DC, T], bf16, kind="ExternalInput").ap()
    w0T = nc.dram_tensor("w0T", [P, DC, F], bf16, kind="ExternalInput").ap()
    w3T = nc.dram_tensor("w3T", [P, FC, D], bf16, kind="ExternalInput").ap()
    out = nc.dram_tensor("out", [T, D], f32, kind="ExternalOutput").ap()

    with tile.TileContext(nc) as tc:
        with (
            tc.tile_pool(name="weights", bufs=1) as wpool,
            tc.tile_pool(name="gelu", bufs=g_bufs) as gpool,
            tc.tile_pool(name="ostage", bufs=o_sb_bufs) as opool,
            tc.tile_pool(name="hps", bufs=h_bufs, space="PSUM") as hpsum,
            tc.tile_pool(name="ops", bufs=n_ops, space="PSUM") as opsum,
        ):
            x_sb = wpool.tile([P, DC, T], bf16, name="x_sb", tag="x_sb")
            w0_sb = wpool.tile([P, DC, F], bf16, name="w0_sb", tag="w0_sb")
            w3_sb = wpool.tile([P, FC, D], bf16, name="w3_sb", tag="w3_sb")

            if dma_scheme == "simple":
                # Load x and w0 first (first h-tile needs ALL d-chunks of
                # both); w3 f-chunks stream in behind.
                for dc in range(DC):
                    for s in range(x_split):
                        w = T // x_split
                        nc.sync.dma_start(x_sb[:, dc, s * w:(s + 1) * w],
                                          xT[:, dc, s * w:(s + 1) * w])
                    for s in range(w0_split):
                        w = F // w0_split
                        nc.sync.dma_start(w0_sb[:, dc, s * w:(s + 1) * w],
                                          w0T[:, dc, s * w:(s + 1) * w])
                for g in range(FC // w3_group):
                    lo, hi = g * w3_group, (g + 1) * w3_group
                    nc.sync.dma_start(w3_sb[:, lo:hi], w3T[:, lo:hi])
            else:
                # Critical-prefix-first: x for tb0, then per-f-group w0 (all
                # d-chunks) and w3 interleaved in the order GEMM1/GEMM2
                # consume them, then the rest of x.
                for dc in range(DC):
                    nc.sync.dma_start(x_sb[:, dc, 0:TBS], xT[:, dc, 0:TBS])
                for g in range(FC // fcg):
                    flo, fhi = g * fcg * P, (g + 1) * fcg * P
                    for dc in range(DC):
                        nc.sync.dma_start(w0_sb[:, dc, flo:fhi],
                                          w0T[:, dc, flo:fhi])
                    nc.sync.dma_start(w3_sb[:, g * fcg:(g + 1) * fcg],
                                      w3T[:, g * fcg:(g + 1) * fcg])
                for tb in range(1, NTBS):
                    for dc in range(DC):
                        nc.sync.dma_start(
                            x_sb[:, dc, tb * TBS:(tb + 1) * TBS],
                            xT[:, dc, tb * TBS:(tb + 1) * TBS])

            for tb in range(NTBS):
                o_ps = [
                    opsum.tile([P, DW], f32, name=f"o_ps_{tb}_{i}", tag="o_ps")
                    for i in range(n_ops)
                ]

                def emit_o(fc, g_sb):
                    for ts in range(NTS):
                        for dc2 in range(2):
                            nc.tensor.matmul(
                                o_ps[ts * 2 + dc2][:],
                                g_sb[:, ts * P:(ts + 1) * P],
                                w3_sb[:, fc, dc2 * DW:(dc2 + 1) * DW],
                                start=(fc == 0),
                                stop=(fc == FC - 1),
                            )

                pending = None
                for fc in range(FC):
                    h_ps = hpsum.tile([P, TBS], f32, name=f"h_ps_{tb}_{fc}", tag="h_ps")
                    for dc in range(DC):
                        nc.tensor.matmul(
                            h_ps[:],
                            w0_sb[:, dc, fc * P:(fc + 1) * P],
                            x_sb[:, dc, tb * TBS:(tb + 1) * TBS],
                            start=(dc == 0),
                            stop=(dc == DC - 1),
                        )
                    g_sb = gpool.tile([P, TBS], bf16, name=f"g_{tb}_{fc}", tag="g")
                    nc.scalar.activation(g_sb[:], h_ps[:], AFT.Gelu)
                    if not pipeline_o:
                        emit_o(fc, g_sb)
                    else:
                        if pending is not None:
                            emit_o(*pending)
                        pending = (fc, g_sb)
                if pending is not None:
                    emit_o(*pending)

                for ts in range(NTS):
                    for dc2 in range(2):
                        o_sb = opool.tile([P, DW], f32, name=f"o_sb_{tb}_{ts}_{dc2}",
                                          tag="o_sb")
                        nc.vector.tensor_copy(o_sb[:], o_ps[ts * 2 + dc2][:])
                        nc.sync.dma_start(
                            out[tb * TBS + ts * P: tb * TBS + (ts + 1) * P,
                                dc2 * DW:(dc2 + 1) * DW],
                            o_sb[:],
                        )

    nc.compile()
    return nc


def _build_nc_v2(
    g_extra=0,           # extra gelu-tile slots beyond FC (lookahead into next block)
    h_bufs=3,
    o_ps_bufs=2,
    o_sb_bufs=3,
    x_bufs=2,
    fcg=4,               # fc per w0/w3 DMA group
    x_coarse=True,       # one DMA per x block vs per-dc
    w0_coarse=False,     # one DMA per w0 f-group vs per-dc
    warmup_mms=8,        # scratch matmuls issued before the real work so the
                         # PE rides out the HAM cold-clock window during the
                         # initial DMA wait instead of during real matmuls
):
    """TB=512 two-phase variant: per 512-token block, phase A runs GEMM1+GELU
    for all 32 f-chunks (g tiles [128,512] bf16 stay in SBUF), phase B runs
    GEMM2 as 8 sequential PSUM accumulation groups (one [t=128, d=512] output
    tile each, contraction over all 32 f-chunks). x is streamed per-block
    instead of fully resident to stay under the SBUF cap."""
    import sys
    if "/opt/trn_rl_repo" not in sys.path:
        sys.path.insert(0, "/opt/trn_rl_repo")
    import concourse.tile as tile
    import concourse.mybir as mybir
    from concourse import bacc

    bf16 = mybir.dt.bfloat16
    f32 = mybir.dt.float32
    AFT = mybir.ActivationFunctionType

    TBS = 512
    NTBS = T // TBS      # 4
    NTS = TBS // P       # 4

    nc = bacc.Bacc(
        "TRN2",
        target_bir_lowering=False,
        debug=False,
        enable_asserts=True,
        num_devices=N_CORES,
    )

    xT = nc.dram_tensor("xT", [P, DC, T], bf16, kind="ExternalInput").ap()
    w0T = nc.dram_tensor("w0T", [P, DC, F], bf16, kind="ExternalInput").ap()
    w3T = nc.dram_tensor("w3T", [P, FC, D], bf16, kind="ExternalInput").ap()
    out = nc.dram_tensor("out", [T, D], f32, kind="ExternalOutput").ap()

    with tile.TileContext(nc) as tc:
        with (
            tc.tile_pool(name="weights", bufs=1) as wpool,
            tc.tile_pool(name="xin", bufs=x_bufs) as xpool,
            tc.tile_pool(name="gelu", bufs=FC + g_extra) as gpool,
            tc.tile_pool(name="ostage", bufs=o_sb_bufs) as opool,
            tc.tile_pool(name="hps", bufs=h_bufs, space="PSUM") as hpsum,
            tc.tile_pool(name="ops", bufs=o_ps_bufs, space="PSUM") as opsum,
        ):
            w0_sb = wpool.tile([P, DC, F], bf16, name="w0_sb", tag="w0_sb")
            w3_sb = wpool.tile([P, FC, D], bf16, name="w3_sb", tag="w3_sb")

            x_tiles = {}
            def load_x(tb):
                xt = xpool.tile([P, DC, TBS], bf16, name=f"x_{tb}", tag="x")
                if x_coarse:
                    nc.sync.dma_start(xt[:], xT[:, :, tb * TBS:(tb + 1) * TBS])
                else:
                    for dc in range(DC):
                        nc.sync.dma_start(xt[:, dc],
                                          xT[:, dc, tb * TBS:(tb + 1) * TBS])
                x_tiles[tb] = xt

            if warmup_mms:
                with (
                    tc.tile_pool(name="warm", bufs=1) as warmpool,
                    tc.tile_pool(name="warmps", bufs=1, space="PSUM") as warmpsum,
                ):
                    wsrc = warmpool.tile([P, DW], bf16, name="wsrc", tag="wsrc")
                    wps = warmpsum.tile([P, DW], f32, name="wps", tag="wps")
                    nc.gpsimd.memset(wsrc[:], 0.0)
                    for i in range(warmup_mms):
                        nc.tensor.matmul(wps[:], wsrc[:, :P], wsrc[:],
                                         start=(i == 0), stop=(i == warmup_mms - 1))

            # critical prefix: x[tb0], then w0/w3 by f-group in consumption order
            load_x(0)
            for g in range(FC // fcg):
                flo, fhi = g * fcg * P, (g + 1) * fcg * P
                if w0_coarse:
                    nc.sync.dma_start(w0_sb[:, :, flo:fhi], w0T[:, :, flo:fhi])
                else:
                    for dc in range(DC):
                        nc.sync.dma_start(w0_sb[:, dc, flo:fhi],
                                          w0T[:, dc, flo:fhi])
                nc.sync.dma_start(w3_sb[:, g * fcg:(g + 1) * fcg],
                                  w3T[:, g * fcg:(g + 1) * fcg])

            for tb in range(NTBS):
                if tb + 1 < NTBS:
                    load_x(tb + 1)
                xt = x_tiles.pop(tb)
                # phase A: GEMM1 + GELU for all fc
                g_tiles = []
                for fc in range(FC):
                    h_ps = hpsum.tile([P, TBS], f32, name=f"h_{tb}_{fc}", tag="h_ps")
                    for dc in range(DC):
                        nc.tensor.matmul(
                            h_ps[:],
                            w0_sb[:, dc, fc * P:(fc + 1) * P],
                            xt[:, dc],
                            start=(dc == 0),
                            stop=(dc == DC - 1),
                        )
                    g_sb = gpool.tile([P, TBS], bf16, name=f"g_{tb}_{fc}", tag="g")
                    nc.scalar.activation(g_sb[:], h_ps[:], AFT.Gelu)
                    g_tiles.append(g_sb)
                # phase B: GEMM2, one [t=128, d=512] accumulation group at a time
                for ts in range(NTS):
                    for dc2 in range(2):
                        o_ps = opsum.tile([P, DW], f32, name=f"o_{tb}_{ts}_{dc2}",
                                          tag="o_ps")
                        for fc in range(FC):
                            nc.tensor.matmul(
                                o_ps[:],
                                g_tiles[fc][:, ts * P:(ts + 1) * P],
                                w3_sb[:, fc, dc2 * DW:(dc2 + 1) * DW],
                                start=(fc == 0),
                                stop=(fc == FC - 1),
                            )
                        o_sb = opool.tile([P, DW], f32, name=f"os_{tb}_{ts}_{dc2}",
                                          tag="o_sb")
                        nc.vector.tensor_copy(o_sb[:], o_ps[:])
                        nc.sync.dma_start(
                            out[tb * TBS + ts * P: tb * TBS + (ts + 1) * P,
                                dc2 * DW:(dc2 + 1) * DW],
                            o_sb[:],
                        )

    nc.compile()
    return nc


def _get_nc():
    if "nc" not in _cache:
        import os
        variant = os.environ.get("KERNEL_VARIANT", "v2")
        _cache["nc"] = _build_nc_v2() if variant == "v2" else _build_nc()
    return _cache["nc"]


def kernel(**inputs):
    import sys
    if "/opt/trn_rl_repo" not in sys.path:
        sys.path.insert(0, "/opt/trn_rl_repo")
    from concourse import bass_utils

    output_tensor = np.asarray(inputs["output_tensor"], dtype=np.float32)  # [1, 8]
    x = np.asarray(inputs["inputs"], dtype=np.float32)   # [1, 8, 2048, 1024]
    w0 = np.asarray(inputs["w0"], dtype=np.float32)      # [8, 4096, 1024]
    w3 = np.asarray(inputs["w3"], dtype=np.float32)      # [8, 1024, 4096]

    bf = ml_dtypes.bfloat16
    in_maps = []
    for e in range(N_CORES):
        # cast to bf16 first (halves bytes moved by the transposes)
        xe = x[0, e].astype(bf)     # [t, d]
        w0e = w0[e].astype(bf)      # [f, d]
        w3e = w3[e].astype(bf)      # [d, f]
        # xT: [d, t] -> [128, 8, 2048]; w0T: [d, f] -> [128, 8, 4096];
        # w3T: [f, d] -> [128, 32, 1024]  (partition = contraction dim % 128)
        in_maps.append({
            "xT": np.ascontiguousarray(xe.T.reshape(DC, P, T).transpose(1, 0, 2)),
            "w0T": np.ascontiguousarray(w0e.T.reshape(DC, P, F).transpose(1, 0, 2)),
            "w3T": np.ascontiguousarray(w3e.T.reshape(FC, P, D).transpose(1, 0, 2)),
        })

    nc = _get_nc()
    res = bass_utils.run_bass_kernel_spmd(nc, in_maps, core_ids=list(range(N_CORES)))
    out_full = np.stack([res.results[e]["out"] for e in range(N_CORES)])[None]

    # unpopular experts with zero gating activity produce zeros
    unpop = output_tensor[:, NUM_LOCAL:].sum(axis=0) != 0
    mask = np.concatenate([np.ones(NUM_LOCAL, dtype=bool), unpop])
    out_full = out_full * mask[None, :, None, None].astype(np.float32)
    return out_full.astype(np.float32)
